# revision 6
# baseline (speedup 1.0000x reference)
"""LN + Linear (no bias) + Sigmoid, tensor-parallel over 8 TRN2 NeuronCores.

Math: y = sigmoid(LN(x) @ W.T), x [8192, 4096] f32, W [16384, 4096] f32.

Decomposition used on device (per core, W sharded along d_out into 2048 cols):
    y[t,o] = sigmoid( r[t] * ( sum_d x[t,d] W[o,d]  -  mean[t] * wsum[o] ) )
with mean[t] = mean_d x[t,d], r[t] = rsqrt(var[t] + eps), wsum[o] = sum_d W[o,d].

So the GEMM runs on RAW x (bf16), the mean subtraction becomes a rank-1
correction (one K=1 matmul accumulated into the same PSUM group, stationary =
-mean[t] row, moving = wsum[o] row), and the 1/std scale is applied by the
ScalarE Sigmoid activation at PSUM eviction (per-partition scale AP).
LN stats are computed on-device from x in natural layout via bn_stats/bn_aggr.

Host-side prep (not part of HW time): transpose/tile x and W into K-major
layouts so every DMA reads contiguous 8KB per-partition lines, cast to bf16,
compute wsum. Host-side post: concat the 8 per-core [8192, 2048] outputs.
"""

import numpy as np
import ml_dtypes

T = 8192        # tokens
D = 4096        # d_in (contraction)
O_FULL = 16384  # d_out
NCORES = 8
OSH = O_FULL // NCORES  # 2048 per-core output shard
P = 128
NK = D // P     # 32 k-tiles
NT = T // P     # 64 token tiles
EPS = 1e-5

_BUILT = None
LAST_RESULTS = None  # BassKernelResults of the most recent run (for test.py)


def _build():
    import concourse.bass as bass
    import concourse.mybir as mybir
    import concourse.tile as tile
    from concourse import bacc
    from concourse.masks import make_identity

    f32 = mybir.dt.float32
    bf16 = mybir.dt.bfloat16

    nc = bacc.Bacc("TRN2", target_bir_lowering=False, debug=False,
                   num_devices=NCORES)

    # xt[i, p, k, t] = x[i*128+t, k*128+p]  (x^T, tiled: 8KB contiguous lines)
    xt_d = nc.dram_tensor("xt", [NT, P, NK, P], bf16, kind="ExternalInput")
    # xn = x natural layout (for LN stats)
    xn_d = nc.dram_tensor("xn", [T, D], bf16, kind="ExternalInput")
    # wt[k, p, o] = W_shard[o, k*128+p]  (W^T, tiled)
    wt_d = nc.dram_tensor("wt", [NK, P, OSH], bf16, kind="ExternalInput")
    # wsum[0, o] = sum_d W_shard[o, d]
    ws_d = nc.dram_tensor("wsum", [1, OSH], bf16, kind="ExternalInput")
    out_d = nc.dram_tensor("out", [T, OSH], f32, kind="ExternalOutput")

    with tile.TileContext(nc) as tc:
        with (
            tc.tile_pool(name="wres", bufs=1) as wres,      # resident W (128KB/part)
            tc.tile_pool(name="const", bufs=1) as const,
            tc.tile_pool(name="xb", bufs=2) as xbpool,      # x^T tile per t-tile
            tc.tile_pool(name="xs", bufs=2) as xspool,      # stats input tile
            tc.tile_pool(name="st", bufs=2) as stpool,      # bn stats scratch
            tc.tile_pool(name="vec", bufs=1) as vecpool,    # r / negmean columns
            tc.tile_pool(name="mrow", bufs=3) as mrowpool,  # transposed -mean rows
            tc.tile_pool(name="ot", bufs=2) as otpool,      # output staging
            tc.tile_pool(name="ps", bufs=3, space="PSUM") as pspool,    # GEMM acc
            tc.tile_pool(name="pst", bufs=2, space="PSUM") as pstpool,  # transposes
        ):
            # ---- constants / resident weights ----
            ident = const.tile([P, P], bf16)
            make_identity(nc, ident[:, :])

            eps_sb = const.tile([P, 1], f32)
            nc.vector.memset(eps_sb[:, :], EPS)

            wsum_sb = const.tile([1, OSH], bf16)
            nc.sync.dma_start(out=wsum_sb[:, :], in_=ws_d[:, :])

            w_sb = const.tile([P, NK, OSH], bf16)
            for k in range(NK):
                nc.sync.dma_start(out=w_sb[:, k, :], in_=wt_d[k])

            r_all = vecpool.tile([P, NT], f32)       # rsqrt(var+eps) per token
            negmean = vecpool.tile([P, NT], bf16)    # -mean per token

            mrows = {}  # t-tile index -> [1,128] bf16 row of -mean

            def emit_stats(i):
                # LN stats for token tile i: mean/var via bn_stats over D=4096
                xs = xspool.tile([P, D], bf16)
                nc.sync.dma_start(out=xs[:, :], in_=xn_d[i * P:(i + 1) * P, :])
                xs3 = xs[:, :].rearrange("p (n f) -> p n f", f=512)
                stats = stpool.tile([P, D // 512, 6], f32)
                for s in range(D // 512):
                    nc.vector.bn_stats(out=stats[:, s, :], in_=xs3[:, s, :])
                mv = stpool.tile([P, 2], f32)
                nc.vector.bn_aggr(out=mv[:, :], in_=stats[:, :, :])
                # r = 1/sqrt(var + eps); negmean = -mean (bf16 for PE)
                std = stpool.tile([P, 1], f32)
                nc.scalar.activation(std[:, :], mv[:, 1:2],
                                     mybir.ActivationFunctionType.Sqrt,
                                     bias=eps_sb[:, :])
                nc.vector.reciprocal(r_all[:, i:i + 1], std[:, :])
                nc.scalar.mul(negmean[:, i:i + 1], mv[:, 0:1], -1.0)

            def emit_transpose(i):
                # [-mean] column i -> [1,128] row (PE transpose, tiny)
                pt = pstpool.tile([1, P], bf16)
                nc.tensor.transpose(pt[:, :], negmean[:, i:i + 1], ident[:, :])
                mr = mrowpool.tile([1, P], bf16)
                nc.vector.tensor_copy(mr[:, :], pt[:, :])
                mrows[i] = mr

            # stats for the first tiles + transposes ahead of the GEMM loop
            emit_stats(0)
            emit_stats(1)
            emit_transpose(0)
            emit_transpose(1)

            for i in range(NT):
                xb = xbpool.tile([P, NK, P], bf16)
                nc.sync.dma_start(out=xb[:, :, :], in_=xt_d[i])
                if i + 2 < NT:
                    emit_stats(i + 2)

                psA = pspool.tile([P, 1024], f32, tag="ps")
                psB = pspool.tile([P, 1024], f32, tag="ps")
                for k in range(NK):
                    lhs = xb[:, k, :]
                    nc.tensor.matmul(psA[:, 0:512], lhs, w_sb[:, k, 0:512],
                                     start=(k == 0), stop=False)
                    nc.tensor.matmul(psA[:, 512:1024], lhs, w_sb[:, k, 512:1024],
                                     start=(k == 0), stop=False)
                for k in range(NK):
                    lhs = xb[:, k, :]
                    nc.tensor.matmul(psB[:, 0:512], lhs, w_sb[:, k, 1024:1536],
                                     start=(k == 0), stop=False)
                    nc.tensor.matmul(psB[:, 512:1024], lhs, w_sb[:, k, 1536:2048],
                                     start=(k == 0), stop=False)

                if i + 2 < NT:
                    emit_transpose(i + 2)

                # rank-1 LN mean correction: += (-mean[t]) * wsum[o]
                mr = mrows.pop(i)
                nc.tensor.matmul(psA[:, 0:512], mr[:, :], wsum_sb[:, 0:512],
                                 start=False, stop=True)
                nc.tensor.matmul(psA[:, 512:1024], mr[:, :], wsum_sb[:, 512:1024],
                                 start=False, stop=True)
                nc.tensor.matmul(psB[:, 0:512], mr[:, :], wsum_sb[:, 1024:1536],
                                 start=False, stop=True)
                nc.tensor.matmul(psB[:, 512:1024], mr[:, :], wsum_sb[:, 1536:2048],
                                 start=False, stop=True)

                # eviction: sigmoid(r[t] * psum) -> SBUF f32 -> DRAM
                ot = otpool.tile([P, OSH], f32)
                nc.scalar.activation(ot[:, 0:1024], psA[:, :],
                                     mybir.ActivationFunctionType.Sigmoid,
                                     scale=r_all[:, i:i + 1])
                nc.scalar.activation(ot[:, 1024:2048], psB[:, :],
                                     mybir.ActivationFunctionType.Sigmoid,
                                     scale=r_all[:, i:i + 1])
                nc.sync.dma_start(out=out_d[i * P:(i + 1) * P, :], in_=ot[:, :])

    nc.compile()
    return nc


def _get_nc():
    global _BUILT
    if _BUILT is None:
        _BUILT = _build()
    return _BUILT


def prepare_in_maps(x, W):
    x = np.asarray(x, dtype=np.float32)
    W = np.asarray(W, dtype=np.float32)
    bf = ml_dtypes.bfloat16

    # xt[i, p, k, t] = x[i*128+t, k*128+p]
    xt = np.ascontiguousarray(
        x.reshape(NT, P, NK, P).transpose(0, 3, 2, 1)).astype(bf)
    xn = x.astype(bf)

    in_maps = []
    for c in range(NCORES):
        Wsh = W[c * OSH:(c + 1) * OSH]                    # [2048, 4096]
        wt = np.ascontiguousarray(Wsh.T).reshape(NK, P, OSH).astype(bf)
        ws = Wsh.sum(axis=1).reshape(1, OSH).astype(bf)
        in_maps.append({"xt": xt, "xn": xn, "wt": wt, "wsum": ws})
    return in_maps


def kernel(x, W):
    global LAST_RESULTS
    from concourse.bass_utils import run_bass_kernel_spmd

    in_maps = prepare_in_maps(x, W)
    nc = _get_nc()
    res = run_bass_kernel_spmd(nc, in_maps, list(range(NCORES)))
    LAST_RESULTS = res
    out = np.concatenate([res.results[c]["out"] for c in range(NCORES)], axis=1)
    return np.ascontiguousarray(out)
